# revision 28
# baseline (speedup 1.0000x reference)
"""GQA attention block (B=2, S=2048, E=2048, H=32, HKV=8, D=64) on 8 trn2 cores.

Sharding: tensor-parallel over heads. Core c owns q-heads 4c..4c+3 and kv-head c.
Each core computes its heads' attention for ALL rows, then an AllToAll exchanges
head-blocks for row-blocks so each core runs the output projection for its own
512-row slice against the full out_w. Host concatenates row slices.

All matmuls run as float32r (TF32-like, full PE rate at N>=512). Softmax is
computed without max-subtraction (scores are O(4), exp cannot overflow), with
denominators obtained by augmenting V with a ones column.

v2: exp split between the scalar engine (true exp, cols 0:SPL) and the vector
engine (Schraudolph bit-trick exp -> int16 bitcast as f16, cols SPL:1024) so
neither engine exceeds the tensor-engine cycle; reciprocal via the ~5x faster
reciprocal_approx_fast; phaseC split into an even-kt pass (overlaps the a2a1
collective) and an odd-kt pass (bias folded into pass 1).

v6: normalization moved out of phaseB into phaseC. The per-j reciprocal +
DRAM-broadcast chain entangled with the a2a collective and stalled every
engine ~28us per collective (plus a PE HAM re-throttle). The a2a now carries
unnormalized head outputs plus denominator rows (130-row payload); den rows
are gathered on the idle gpsimd DMA ring during compute, and phaseC does one
batched reciprocal + broadcast-multiply into the packed oTown tile per half.
"""

import numpy as np

B, S, E = 2, 2048, 2048
H, HKV, D = 32, 8, 64
NCORES = 8
ROWS = B * S              # 4096
RPC = ROWS // NCORES      # 512 output rows per core
HQ = H // NCORES          # 4 q heads per core
QCOLS = HQ * D            # 256
NCH = ROWS // 512         # 8 row chunks
NKT = E // 128            # 16 k-tiles over E
SKT = S // 128            # 16 key tiles per batch

_CACHE = {}
_VONES = np.zeros((128, B * (S // 128) * 128), dtype=np.float16)
_VONES[:, 64::128] = 1.0


def _build_module():
    from contextlib import ExitStack

    import concourse.tile as tile
    from concourse import bacc, mybir

    dt = mybir.dt
    f32, f32r, bf16 = dt.float32, dt.float32r, dt.bfloat16
    f16 = dt.float16
    i16 = dt.int16
    EXP = mybir.ActivationFunctionType.Exp
    MULT = mybir.AluOpType.mult
    ADD = mybir.AluOpType.add

    # ACT computes true exp on ex[:, 0:SPL]; DVE computes Schraudolph bit-trick
    # exp on ex[:, SPL:1024] (i16 = raw_score*0.125*1477.32 + 15308, bitcast f16;
    # max rel err ~3.5%). Split keeps both engines under the tensor-engine cycle.
    SPL = 640
    SCH_A = 0.125 * 1477.3196
    SCH_B = 15308.0

    nc = bacc.Bacc("TRN2", target_bir_lowering=False, debug=False, num_devices=NCORES)

    xT = nc.dram_tensor("xT", [E, ROWS], f16, kind="ExternalInput")
    wq = nc.dram_tensor("wq", [128, NKT * QCOLS], f16, kind="ExternalInput")
    wkv = nc.dram_tensor("wkv", [128, NKT * 128], f16, kind="ExternalInput")
    cosE = nc.dram_tensor("cosE", [128, S], f32, kind="ExternalInput")
    sinE = nc.dram_tensor("sinE", [128, S], f32, kind="ExternalInput")
    perm = nc.dram_tensor("perm", [128, 128], f16, kind="ExternalInput")
    ident = nc.dram_tensor("ident", [128, 64], f16, kind="ExternalInput")
    sel = nc.dram_tensor("sel", [16, NCORES * 128], f16, kind="ExternalInput")
    outw = nc.dram_tensor("outw", [E, E], f16, kind="ExternalInput")
    biasr = nc.dram_tensor("biasr", [128, E], f32, kind="ExternalInput")
    vones = nc.dram_tensor("vones", [128, B * SKT * 128], f16, kind="ExternalInput")
    out = nc.dram_tensor("out", [RPC, E], f32, kind="ExternalOutput")

    with tile.TileContext(nc) as tc, ExitStack() as ctx:
        persist = ctx.enter_context(tc.tile_pool(name="persist", bufs=1))
        dram = ctx.enter_context(tc.tile_pool(name="dram", bufs=1, space="DRAM"))

        qT0 = persist.tile([128, ROWS], f16, tag="qT0")  # heads 0,1 (local), D-major
        qT1 = persist.tile([128, ROWS], f16, tag="qT1")  # heads 2,3
        kT2 = persist.tile([128, ROWS], f16, tag="kT2")  # roped kT duplicated on 0:64 / 64:128
        vsb = persist.tile([128, B * SKT * 128], f16, tag="vsb")  # rows-major v + ones col
        perm_sb = persist.tile([128, 128], f16, tag="perm")
        ident_sb = persist.tile([128, 64], f16, tag="ident")
        sel_sb = persist.tile([16, NCORES * 128], f16, tag="sel")

        nc.sync.dma_start(perm_sb[:], perm[:])
        nc.sync.dma_start(ident_sb[:], ident[:])
        nc.sync.dma_start(sel_sb[:], sel[:])
        # ones columns of the augmented-V tile (data columns overwritten later)
        nc.sync.dma_start(vsb[:], vones[:])

        # 130 rows: 0:64 head A, 64:128 head B (unnormalized), 128 denA, 129 denB.
        # v8: evacuation packs both heads into ONE [128,512] tile + dens into a
        # [2,512] tile so each j ships as 2 DMAs instead of 4 (the sync
        # sequencer costs ~600ns of DIRECT2D descriptor processing per DMA).
        a2aA_in = dram.tile([NCORES, 130, RPC], f16, tag="a2aA_in")
        a2aA_out = dram.tile([NCORES, 130, RPC], f16, tag="a2aA_out")
        a2aB_in = dram.tile([NCORES, 130, RPC], f16, tag="a2aB_in")
        a2aB_out = dram.tile([NCORES, 130, RPC], f16, tag="a2aB_out")
        rdCA = dram.tile([16, 512], f32, tag="rdCA")
        rdCB = dram.tile([16, 512], f32, tag="rdCB")

        # ---------------- Phase A: QKV projections + RoPE + V transpose -------------
        with ExitStack() as ctxA, nc.named_scope("phaseA"):
            wpool = ctxA.enter_context(tc.tile_pool(name="wpool", bufs=1))
            xpool = ctxA.enter_context(tc.tile_pool(name="xpool", bufs=32))
            cspool = ctxA.enter_context(tc.tile_pool(name="cspool", bufs=2))
            tmpA = ctxA.enter_context(tc.tile_pool(name="tmpA", bufs=2))
            psA1 = ctxA.enter_context(tc.tile_pool(name="psA1", bufs=2, space="PSUM"))
            psA2 = ctxA.enter_context(tc.tile_pool(name="psA2", bufs=1, space="PSUM"))

            wq_sb = wpool.tile([128, NKT * QCOLS], f16, tag="wq")
            wkv_sb = wpool.tile([128, NKT * 128], f16, tag="wkv")
            nc.sync.dma_start(wq_sb[:], wq[:])
            nc.sync.dma_start(wkv_sb[:], wkv[:])

            for chp in range(NCH // 2):
                ps1k = slice(chp * 1024, (chp + 1) * 1024)
                xts = []
                for kt in range(NKT):
                    xt = xpool.tile([128, 1024], f16, tag="xt")
                    nc.sync.dma_start(xt[:], xT[kt * 128 : (kt + 1) * 128, ps1k])
                    xts.append(xt)
                for sub in range(2):
                    ch = chp * 2 + sub
                    cs = slice(ch * 512, (ch + 1) * 512)
                    ss = slice(sub * 512, (sub + 1) * 512)
                    q0_ps = psA1.tile([128, 512], f32, tag="q0")
                    q1_ps = psA1.tile([128, 512], f32, tag="q1")
                    kv_ps = psA1.tile([128, 512], f32, tag="kv")
                    for kt in range(NKT):
                        st, sp = kt == 0, kt == NKT - 1
                        xs = xts[kt][:, ss]
                        wqk = wq_sb[:, kt * QCOLS : kt * QCOLS + 128]
                        wqk2 = wq_sb[:, kt * QCOLS + 128 : kt * QCOLS + 256]
                        nc.tensor.matmul(q0_ps[:], wqk, xs, start=st, stop=sp)
                        nc.tensor.matmul(q1_ps[:], wqk2, xs, start=st, stop=sp)
                        nc.tensor.matmul(
                            kv_ps[:], wkv_sb[:, kt * 128 : (kt + 1) * 128], xs,
                            start=st, stop=sp,
                        )

                    # PSUM -> SBUF (rounds to f32r)
                    nc.scalar.copy(qT0[:, cs], q0_ps[:])
                    nc.scalar.copy(qT1[:, cs], q1_ps[:])
                    nc.scalar.copy(kT2[0:64, cs], kv_ps[0:64, :])
                    vtt = tmpA.tile([128, 512], f16, tag="vtt")
                    nc.scalar.copy(vtt[64:128, :], kv_ps[64:128, :])

                    # RoPE: t = t*cosE + (perm @ t)*sinE   (in place)
                    scs = slice((ch % 4) * 512, (ch % 4 + 1) * 512)  # pos = row % S
                    cos_sb = cspool.tile([128, 512], f32, tag="cos")
                    sin_sb = cspool.tile([128, 512], f32, tag="sin")
                    nc.sync.dma_start(cos_sb[:], cosE[:, scs])
                    nc.sync.dma_start(sin_sb[:], sinE[:, scs])
                    for t, p in ((qT0, 128), (qT1, 128), (kT2, 64)):
                        rot_ps = psA2.tile([128, 512], f32, tag="rot")
                        nc.tensor.matmul(
                            rot_ps[0:p, :], perm_sb[0:p, 0:p], t[0:p, cs],
                            start=True, stop=True,
                        )
                        tmp = tmpA.tile([128, 512], f32, tag="ropetmp")
                        nc.vector.scalar_tensor_tensor(
                            out=tmp[0:p, :], in0=rot_ps[0:p, :], scalar=1.0,
                            in1=sin_sb[0:p, :], op0=MULT, op1=MULT,
                        )
                        nc.vector.scalar_tensor_tensor(
                            out=t[0:p, cs], in0=t[0:p, cs], scalar=1.0,
                            in1=cos_sb[0:p, :], op0=MULT, op1=MULT,
                        )
                        nc.vector.scalar_tensor_tensor(
                            out=t[0:p, cs], in0=t[0:p, cs], scalar=1.0,
                            in1=tmp[0:p, :], op0=MULT, op1=ADD,
                        )
                    # duplicate roped k on partitions 64:128 (for row-group packing)
                    nc.sync.dma_start(kT2[64:128, cs], kT2[0:64, cs])

                    # V transpose: [64,512] (keys on free) -> 4x [128,64] rows-major
                    b = ch // 4
                    for j in range(4):
                        kt_key = (ch % 4) * 4 + j
                        v_ps = psA2.tile([128, 64], f16, tag="vps")
                        nc.tensor.transpose(
                            v_ps[:], vtt[64:128, j * 128 : (j + 1) * 128],
                            ident_sb[64:128, :],
                        )
                        blk = (b * SKT + kt_key) * 128
                        nc.vector.tensor_copy(vsb[:, blk : blk + 64], v_ps[:])

        # ---------------- Phase C pools opened early so out_w prefetch overlaps B ---
        ctxC = ctx.enter_context(ExitStack())
        cpool = ctxC.enter_context(tc.tile_pool(name="cpool", bufs=1))
        wcolpE = ctxC.enter_context(tc.tile_pool(name="wcolpE", bufs=32))
        wcolpO = ctxC.enter_context(tc.tile_pool(name="wcolpO", bufs=32))
        evpool = ctxC.enter_context(tc.tile_pool(name="evpool", bufs=16))
        obuf = ctxC.enter_context(tc.tile_pool(name="obuf", bufs=4))
        otraw = ctxC.enter_context(tc.tile_pool(name="otraw", bufs=6))
        rbtp = ctxC.enter_context(tc.tile_pool(name="rbtp", bufs=4))
        denp = ctxC.enter_context(tc.tile_pool(name="denp", bufs=1))
        oTown = cpool.tile([128, NKT * RPC], f16, tag="oTown")
        bias_sb = cpool.tile([128, E], f32, tag="bias")
        den16A = cpool.tile([16, 512], f16, tag="den16A")  # rows 2j+hh = den(j,hh)
        den16B = cpool.tile([16, 512], f16, tag="den16B")
        nc.sync.dma_start(bias_sb[:], biasr[:])
        # out_w column tiles: BOTH halves prefetched before phaseB. The odd tiles
        # were previously DMA'd after norm_half(B), head-of-line blocked on the
        # sync queue behind rbt DMAs that wait on the a2a1 collective -> ~12us of
        # exposed tail. They only depend on DRAM outw, so fetch them up front.
        EVEN_KT = list(range(0, NKT, 2))
        ODD_KT = list(range(1, NKT, 2))
        wcE = {}
        wcO = {}
        for nch in range(4):
            ns = slice(nch * 512, (nch + 1) * 512)
            for kt in EVEN_KT:
                wc = wcolpE.tile([128, 512], f16, tag="wce")
                nc.sync.dma_start(wc[:], outw[kt * 128 : (kt + 1) * 128, ns])
                wcE[(nch, kt)] = wc
            for kt in ODD_KT:
                wc = wcolpO.tile([128, 512], f16, tag="wco")
                nc.sync.dma_start(wc[:], outw[kt * 128 : (kt + 1) * 128, ns])
                wcO[(nch, kt)] = wc

        # ---------------- Phase B: attention (scoresT -> exp -> A@V) ----------------
        # v7: two j-blocks processed interleaved, AV pipelined one kt behind the
        # scores. The v6 per-j serial chain score->EXP->AV left the PE micro-idle
        # every kt (HAM re-throttle; AV matmuls measured at cold-rate ~420ns) and
        # exposed the ACT EXP latency. Interleaving gives the PE the sibling
        # block's matmuls to chew on during EXP/DVE latency. PSUM: sc(j0), sc(j1)
        # single-buffered + oT(j0), oT(j1) = exactly 8 banks.
        with ExitStack() as ctxB, nc.named_scope("phaseB"):
            expool = ctxB.enter_context(tc.tile_pool(name="expool", bufs=4))
            onorm = ctxB.enter_context(tc.tile_pool(name="onorm", bufs=4))
            psB = ctxB.enter_context(tc.tile_pool(name="psB", bufs=1, space="PSUM"))
            psO = ctxB.enter_context(tc.tile_pool(name="psO", bufs=1, space="PSUM"))

            for hp, qTt in ((0, qT0), (1, qT1)):
                a2a_buf = a2aA_in if hp == 0 else a2aB_in
                for jp in range(0, NCORES, 2):
                    js = (jp, jp + 1)  # same batch: j//4 identical within a pair
                    b = jp // 4

                    # norm-A interleaved into hp1's LAST pair: one slot per kt.
                    # den16A lands ~collectiveA-end (+30us margin); DVE has
                    # ~440ns/kt slack to absorb the MULTs; broadcast DMAs ride
                    # the idle gpsimd ring so the sync queue keeps ship slots.
                    norm_slots = []
                    if hp == 1 and jp == NCORES - 2:
                        orwAs, rbtAs = {}, {}

                        def mk_head():
                            def f():
                                denfA = denp.tile([16, 512], f32, tag="denfA",
                                                  name="denfA")
                                nc.vector.tensor_copy(denfA[:], den16A[:])
                                recsA = denp.tile([16, 512], f32, tag="recsA",
                                                  name="recsA")
                                nc.vector.reciprocal_approx_fast(
                                    out=recsA[:], in_=denfA[:]
                                )
                                nc.sync.dma_start(rdCA[:], recsA[:])
                            return f

                        def mk_dma(jj):
                            def f():
                                orw = otraw.tile([128, 512], f16, tag="orwA",
                                                 name="orwA")
                                nc.sync.dma_start(orw[:], a2aA_out[jj, 0:128, :])
                                rbt = rbtp.tile([128, 512], f32, tag="rbtA",
                                                name="rbtA")
                                nc.gpsimd.dma_start(
                                    rbt[0:64, :],
                                    rdCA[2 * jj : 2 * jj + 1, :].to_broadcast(
                                        (64, 512)
                                    ),
                                )
                                nc.gpsimd.dma_start(
                                    rbt[64:128, :],
                                    rdCA[2 * jj + 1 : 2 * jj + 2, :].to_broadcast(
                                        (64, 512)
                                    ),
                                )
                                orwAs[jj], rbtAs[jj] = orw, rbt
                            return f

                        def mk_mult(jj):
                            kt = EVEN_KT[jj]

                            def f():
                                nc.vector.scalar_tensor_tensor(
                                    out=oTown[:, kt * RPC : (kt + 1) * RPC],
                                    in0=orwAs[jj][:], scalar=1.0,
                                    in1=rbtAs[jj][:], op0=MULT, op1=MULT,
                                )
                            return f

                        norm_slots.append(mk_head())
                        sched = {}
                        for jj in range(NCORES):
                            sched.setdefault(1 + jj, []).append(mk_dma(jj))
                            sched.setdefault(3 + jj, []).append(mk_mult(jj))
                        for slot in range(1, max(sched) + 1):
                            norm_slots.append(
                                (lambda fs: lambda: [f() for f in fs])(
                                    sched.get(slot, [])
                                )
                            )
                    qss = [
                        slice(b * S + (j % 4) * 512, b * S + (j % 4 + 1) * 512)
                        for j in js
                    ]
                    oT = [
                        psO.tile([128, 1024], f32, tag=f"oT{dj}", name=f"oT{dj}")
                        for dj in range(2)
                    ]
                    scp = [
                        psB.tile([128, 1024], f32, tag=f"sc{dj}", name=f"sc{dj}")
                        for dj in range(2)
                    ]
                    exs = [None, None]

                    def issue_av(dj, kt):
                        blk = (b * SKT + kt) * 128
                        st, sp = kt == 0, kt == SKT - 1
                        nc.tensor.matmul(
                            oT[dj][:, 0:512], vsb[:, blk : blk + 128],
                            exs[dj][:, 0:512], start=st, stop=sp,
                        )
                        nc.tensor.matmul(
                            oT[dj][:, 512:1024], vsb[:, blk : blk + 128],
                            exs[dj][:, 512:1024], start=st, stop=sp,
                        )

                    nexs = [None, None]
                    for kt in range(SKT):
                        ks = slice(b * S + kt * 128, b * S + (kt + 1) * 128)
                        for dj in range(2):
                            sc = scp[dj]
                            nc.tensor.matmul(
                                sc[:, 0:512], kT2[0:64, ks], qTt[0:64, qss[dj]],
                                start=True, stop=True,
                            )
                            nc.tensor.matmul(
                                sc[:, 512:1024], kT2[64:128, ks],
                                qTt[64:128, qss[dj]], start=True, stop=True,
                            )
                            nex = expool.tile([128, 1024], f16, tag="ex", name="ex")
                            nc.scalar.activation(
                                nex[:, 0:SPL], sc[:, 0:SPL], EXP, scale=0.125
                            )
                            nc.vector.tensor_scalar(
                                nex[:, SPL:1024].bitcast(i16), sc[:, SPL:1024],
                                SCH_A, SCH_B, MULT, ADD,
                            )
                            nexs[dj] = nex
                        if kt > 0:
                            issue_av(0, kt - 1)
                            issue_av(1, kt - 1)
                        exs[0], exs[1] = nexs[0], nexs[1]
                        if norm_slots:
                            norm_slots.pop(0)()
                    issue_av(0, SKT - 1)
                    issue_av(1, SKT - 1)

                    # ship unnormalized oT + denominator rows; normalize later
                    for dj, j in enumerate(js):
                        onD = onorm.tile([128, 512], f16, tag="onD")
                        nc.scalar.copy(onD[0:64, :], oT[dj][0:64, 0:512])
                        nc.vector.tensor_copy(onD[64:128, :], oT[dj][0:64, 512:1024])
                        onDen = onorm.tile([1, 1024], f16, tag="onDen")
                        nc.scalar.copy(onDen[0:1, 0:512], oT[dj][64:65, 0:512])
                        nc.vector.tensor_copy(
                            onDen[0:1, 512:1024], oT[dj][64:65, 512:1024]
                        )
                        nc.sync.dma_start(a2a_buf[j, 0:128, :], onD[:])
                        nc.sync.dma_start(
                            a2a_buf[j, 128:130, :]
                            .rearrange("two c -> (two c)")
                            .unsqueeze(0),
                            onDen[0:1, :],
                        )
                with nc.named_scope(f"a2a{hp}"):
                    nc.gpsimd.collective_compute(
                        "AllToAll",
                        mybir.AluOpType.bypass,
                        replica_groups=[list(range(NCORES))],
                        ins=[(a2aA_in if hp == 0 else a2aB_in).opt()],
                        outs=[(a2aA_out if hp == 0 else a2aB_out).opt()],
                    )
                # denominator gathers ride the otherwise-idle gpsimd DMA ring:
                # they run the moment the collective data lands without blocking
                # sync-queue DMAs phaseB still needs (head-of-line hazard).
                src_t = a2aA_out if hp == 0 else a2aB_out
                den_t = den16A if hp == 0 else den16B
                for jj in range(NCORES):
                    nc.gpsimd.dma_start(
                        den_t[2 * jj : 2 * jj + 2, :], src_t[jj, 128:130, :]
                    )

        # ---------------- Phase C: output projection for own row slice --------------
        # Pass 1 (even kt, a2aA data) runs while the a2a1 collective is in flight;
        # pass 2 (odd kt, a2aB data) is the only post-a2a1 work. Bias is folded
        # into pass 1 so pass 2 is a single add.
        with nc.named_scope("phaseC"):
            psC = ctxC.enter_context(tc.tile_pool(name="psC", bufs=3, space="PSUM"))
            psR = ctxC.enter_context(tc.tile_pool(name="psR", bufs=2, space="PSUM"))

            ev_tiles = {}
            for nch in range(4):
                ns = slice(nch * 512, (nch + 1) * 512)
                for mt in range(4):
                    acc = psC.tile([128, 512], f32, tag="acc")
                    for i, kt in enumerate(EVEN_KT):
                        nc.tensor.matmul(
                            acc[:],
                            oTown[:, kt * RPC + mt * 128 : kt * RPC + (mt + 1) * 128],
                            wcE[(nch, kt)][:],
                            start=(i == 0), stop=(i == len(EVEN_KT) - 1),
                        )
                    ev = evpool.tile([128, 512], f16, tag="ev")
                    nc.vector.scalar_tensor_tensor(
                        out=ev[:], in0=acc[:], scalar=1.0,
                        in1=bias_sb[:, ns], op0=MULT, op1=ADD,
                    )
                    ev_tiles[(nch, mt)] = ev

            # norm-B: no DRAM bounce. recs broadcast to 128 partitions via a tiny
            # selector matmul into PSUM (free after phaseB); odd orw loads split
            # across the sync + gpsimd queues to halve descriptor serialization.
            denfB = denp.tile([16, 512], f32, tag="denfB")
            nc.vector.tensor_copy(denfB[:], den16B[:])
            recsB32 = denp.tile([16, 512], f32, tag="recsB32")
            nc.vector.reciprocal_approx_fast(out=recsB32[:], in_=denfB[:])
            recsB = denp.tile([16, 512], f16, tag="recsB")
            nc.vector.tensor_copy(recsB[:], recsB32[:])
            for jj, kt in enumerate(ODD_KT):
                orw = otraw.tile([128, 512], f16, tag="orwB", name="orwB")
                q = nc.sync if jj % 2 == 0 else nc.gpsimd
                q.dma_start(orw[:], a2aB_out[jj, 0:128, :])
                rbt_ps = psR.tile([128, 512], f32, tag="rbtB", name="rbtB")
                nc.tensor.matmul(
                    rbt_ps[:], sel_sb[:, jj * 128 : (jj + 1) * 128], recsB[:],
                    start=True, stop=True,
                )
                nc.vector.scalar_tensor_tensor(
                    out=oTown[:, kt * RPC : (kt + 1) * RPC], in0=orw[:],
                    scalar=1.0, in1=rbt_ps[:], op0=MULT, op1=MULT,
                )
            for nch in range(4):
                ns = slice(nch * 512, (nch + 1) * 512)
                for mt in range(4):
                    acc = psC.tile([128, 512], f32, tag="acc")
                    for i, kt in enumerate(ODD_KT):
                        nc.tensor.matmul(
                            acc[:],
                            oTown[:, kt * RPC + mt * 128 : kt * RPC + (mt + 1) * 128],
                            wcO[(nch, kt)][:],
                            start=(i == 0), stop=(i == len(ODD_KT) - 1),
                        )
                    ob = obuf.tile([128, 512], f32, tag="ob")
                    nc.vector.scalar_tensor_tensor(
                        out=ob[:], in0=acc[:], scalar=1.0,
                        in1=ev_tiles[(nch, mt)][:], op0=MULT, op1=ADD,
                    )
                    nc.sync.dma_start(out[mt * 128 : (mt + 1) * 128, ns], ob[:])

    nc.finalize()
    return nc


def _prep_inputs(x, freqs_cos, freqs_sin, wq, wk, wv, out_w, out_b):
    x2 = np.ascontiguousarray(np.asarray(x, dtype=np.float32).reshape(ROWS, E))
    xT = np.ascontiguousarray(x2.T.astype(np.float16))

    cos = np.asarray(freqs_cos, dtype=np.float32).reshape(S, D // 2)
    sin = np.asarray(freqs_sin, dtype=np.float32).reshape(S, D // 2)
    cos_exp = np.repeat(cos.T, 2, axis=0)            # [64, S]
    sin_exp = np.repeat(sin.T, 2, axis=0)
    sin_exp[0::2] *= -1.0                            # -sin on even rows
    cosE = np.ascontiguousarray(np.tile(cos_exp, (2, 1)))  # [128, S]
    sinE = np.ascontiguousarray(np.tile(sin_exp, (2, 1)))

    perm = np.zeros((128, 128), dtype=np.float16)
    idx = np.arange(64)
    perm[2 * idx, 2 * idx + 1] = 1.0
    perm[2 * idx + 1, 2 * idx] = 1.0

    ident = np.tile(np.eye(64, dtype=np.float16), (2, 1))  # [128, 64]

    # selector for the norm-B reciprocal broadcast: sel[:, jj*128:(jj+1)*128].T
    # @ recs puts recs row 2jj on partitions 0:64 and row 2jj+1 on 64:128
    sel_np = np.zeros((16, NCORES * 128), dtype=np.float16)
    for jj in range(NCORES):
        sel_np[2 * jj, jj * 128 : jj * 128 + 64] = 1.0
        sel_np[2 * jj + 1, jj * 128 + 64 : (jj + 1) * 128] = 1.0

    wq_f = np.asarray(wq, dtype=np.float32)
    wk_f = np.asarray(wk, dtype=np.float32)
    wv_f = np.asarray(wv, dtype=np.float32)
    outw_f = np.ascontiguousarray(np.asarray(out_w, dtype=np.float32).astype(np.float16))
    biasr = np.ascontiguousarray(
        np.tile(np.asarray(out_b, dtype=np.float32)[None, :], (128, 1))
    )

    in_maps = []
    for c in range(NCORES):
        wq_c = np.ascontiguousarray(
            wq_f[:, c * QCOLS : (c + 1) * QCOLS]
            .reshape(NKT, 128, QCOLS).transpose(1, 0, 2).reshape(128, NKT * QCOLS)
            .astype(np.float16)
        )
        wkv_c = np.ascontiguousarray(
            np.concatenate(
                [wk_f[:, c * 64 : (c + 1) * 64], wv_f[:, c * 64 : (c + 1) * 64]],
                axis=1,
            ).reshape(NKT, 128, 128).transpose(1, 0, 2).reshape(128, NKT * 128)
            .astype(np.float16)
        )
        in_maps.append(
            {
                "xT": xT, "wq": wq_c, "wkv": wkv_c, "cosE": cosE, "sinE": sinE,
                "perm": perm, "ident": ident, "sel": sel_np, "outw": outw_f,
                "biasr": biasr, "vones": _VONES,
            }
        )
    return in_maps


def kernel(
    x, start_pos, freqs_cos, freqs_sin, wq, wk, wv, out_w, out_b,
    k_cache=None, v_cache=None, _trace=False, _trace_cores=None,
):
    from concourse.bass_utils import run_bass_kernel_spmd

    sp = int(np.asarray(start_pos))
    assert sp == 0, f"kernel specialized for start_pos=0, got {sp}"

    if "nc" not in _CACHE:
        _CACHE["nc"] = _build_module()
    nc = _CACHE["nc"]

    in_maps = _prep_inputs(x, freqs_cos, freqs_sin, wq, wk, wv, out_w, out_b)

    kwargs = {}
    if _trace:
        _install_ntff_hook()
        kwargs = {"trace": True, "trace_cores": _trace_cores}
    res = run_bass_kernel_spmd(nc, in_maps, list(range(NCORES)), **kwargs)

    full = np.concatenate([res.results[c]["out"] for c in range(NCORES)], axis=0)
    out = full.reshape(B, S, E).astype(np.float32)
    if _trace:
        return out, res
    return out


def _install_ntff_hook():
    """The agent image lacks antenv.axon_hooks; synthesize it so trace=True works."""
    import sys, types

    if "antenv.axon_hooks" in sys.modules:
        return
    try:
        from trn_agent_boot.trn_boot import _ntff_profile_via_ctypes

        hook = _ntff_profile_via_ctypes("/opt/axon/libaxon_pjrt.so")
    except Exception:
        hook = None
    mod = types.ModuleType("antenv.axon_hooks")
    mod.get_axon_ntff_profile_hook = lambda: hook
    sys.modules["antenv.axon_hooks"] = mod

